# revision 2
# baseline (speedup 1.0000x reference)
"""Trainium2 Bass kernel for nn_Interaction_GraphConvolution (GNN message passing).

Math (N=2048, F_IN=128, F=64):
    H = X @ W + b                                      # [N, F]
    out[j,f] = sum_k mf[j,k] * H[k,f] * G_k[j,f]
    G_k[j,f] = sum_i A[j,i] * H[i,f] * mh[i,k]         # one [N,N]@[N,F] matmul per k

Sharding: k axis split across 8 cores (256 k's each). Each core holds A and H
(replicated) plus its mh/mf column shards, computes the partial sum over its k
slice, and the host adds the 8 partials.

Per-core schedule:
  - H = X@W+b on device (PE transposes X tiles, bf16 matmul, fp32 psum).
  - A^T materialized once in SBUF as bf16 via 256 PE transposes (lhsT tiles).
  - k's processed in chunks of KB=8 (512 matmul columns = 8 k's x 64 f):
      R[i,(k,f)] = H[i,f]*mh[i,k]      one DVE op w/ step-0 broadcast APs
      G = A @ R                        16 accumulating bf16 matmuls -> fp32 psum
      acc += G * mf[:,k] * Hrow[k,f]   3 DVE ops; Hrow broadcast via ones-matmul
  - Final: reduce acc over the 8 k-chunk slots, DMA out.
"""

import numpy as np

import concourse.bacc as bacc
import concourse.mybir as mybir
from concourse.tile import TileContext
from concourse.masks import make_identity
from concourse.bass_utils import run_bass_kernel_spmd

N = 2048
FIN = 128
F = 64
P = 128
NCORES = 8
KSH = N // NCORES          # 256 k's per core
KB = 8                     # k's per chunk (512 matmul cols)
NKB = KSH // KB            # 32 chunks per core
NIT = N // P               # 16 i tiles
NJT = N // P               # 16 j tiles
NCOL = KB * F              # 512

_CACHE = {}


def _build():
    dt = mybir.dt
    nc = bacc.Bacc("TRN2")

    x_in = nc.declare_dram_parameter("x", [N, FIN], dt.float32, isOutput=False)
    xs_in = nc.declare_dram_parameter("xs", [KSH, FIN], dt.float32, isOutput=False)
    w_in = nc.declare_dram_parameter("w", [FIN, F], dt.float32, isOutput=False)
    b_in = nc.declare_dram_parameter("b", [1, F], dt.float32, isOutput=False)
    a_in = nc.declare_dram_parameter("a", [N, N], dt.float32, isOutput=False)
    mh_in = nc.declare_dram_parameter("mh", [N, KSH], dt.float32, isOutput=False)
    mf_in = nc.declare_dram_parameter("mf", [N, KSH], dt.float32, isOutput=False)
    out_p = nc.declare_dram_parameter("out_p", [N, F], dt.float32, isOutput=True)

    hsh_dram = nc.dram_tensor("hsh_flat", [1, KSH * F], dt.float32)

    with TileContext(nc) as tc:
        with (
            tc.tile_pool(name="const", bufs=1) as cpool,
            tc.tile_pool(name="stage", bufs=2) as stage,
            tc.tile_pool(name="work", bufs=1) as work,
            tc.tile_pool(name="rp", bufs=2) as rp,
            tc.tile_pool(name="tmp", bufs=3) as tmp,
            tc.tile_pool(name="hk", bufs=2) as hkp,
            tc.tile_pool(name="psg", bufs=4, space="PSUM") as psg,
            tc.tile_pool(name="psm", bufs=2, space="PSUM") as psm,
        ):
            ident = cpool.tile([P, P], dt.float32)
            make_identity(nc, ident)
            ones = cpool.tile([1, P], dt.float32)
            nc.any.memset(ones, 1.0)

            # ---- weights / bias ----
            w_sb = cpool.tile([FIN, F], dt.float32)
            nc.sync.dma_start(out=w_sb, in_=w_in[:, :])
            w_bf = cpool.tile([FIN, F], dt.bfloat16)
            nc.any.tensor_copy(out=w_bf, in_=w_sb)
            b_sb = cpool.tile([1, F], dt.float32)
            nc.sync.dma_start(out=b_sb, in_=b_in[:, :])

            # ---- H = X @ W + b  (16 tiles) ----
            h_sb = [cpool.tile([P, F], dt.float32, tag=f"h{i}", name=f"h{i}") for i in range(NIT)]
            for i in range(NIT):
                x_st = stage.tile([P, FIN], dt.float32, tag="xst")
                nc.sync.dma_start(out=x_st, in_=x_in[i * P:(i + 1) * P, :])
                xt_ps = psm.tile([P, P], dt.float32, tag="m")
                nc.tensor.transpose(xt_ps, x_st, ident)
                xt_bf = stage.tile([P, P], dt.bfloat16, tag="xtbf")
                nc.any.tensor_copy(out=xt_bf, in_=xt_ps)
                h_ps = psm.tile([P, F], dt.float32, tag="m")
                nc.tensor.matmul(h_ps, xt_bf, w_bf, start=True, stop=False)
                nc.tensor.matmul(h_ps, ones, b_sb, start=False, stop=True)
                nc.any.tensor_copy(out=h_sb[i], in_=h_ps)

            # ---- Hsh = X[kshard] @ W + b, flattened to DRAM ----
            for t in range(KSH // P):
                xs_st = stage.tile([P, FIN], dt.float32, tag="xst")
                nc.sync.dma_start(out=xs_st, in_=xs_in[t * P:(t + 1) * P, :])
                xst_ps = psm.tile([P, P], dt.float32, tag="m")
                nc.tensor.transpose(xst_ps, xs_st, ident)
                xst_bf = stage.tile([P, P], dt.bfloat16, tag="xtbf")
                nc.any.tensor_copy(out=xst_bf, in_=xst_ps)
                hs_ps = psm.tile([P, F], dt.float32, tag="m")
                nc.tensor.matmul(hs_ps, xst_bf, w_bf, start=True, stop=False)
                nc.tensor.matmul(hs_ps, ones, b_sb, start=False, stop=True)
                hs_sb = stage.tile([P, F], dt.float32, tag="hs")
                nc.any.tensor_copy(out=hs_sb, in_=hs_ps)
                nc.sync.dma_start(
                    out=hsh_dram[0:1, t * P * F:(t + 1) * P * F], in_=hs_sb
                )

            # ---- mh (bf16) / mf (fp32) shards ----
            mh_sb = []
            mf_sb = []
            for i in range(NIT):
                m_st = stage.tile([P, KSH], dt.float32, tag="mst")
                nc.sync.dma_start(out=m_st, in_=mh_in[i * P:(i + 1) * P, :])
                mh_t = work.tile([P, KSH], dt.bfloat16, tag=f"mh{i}")
                nc.any.tensor_copy(out=mh_t, in_=m_st)
                mh_sb.append(mh_t)
                mf_t = work.tile([P, KSH], dt.float32, tag=f"mf{i}")
                nc.sync.dma_start(out=mf_t, in_=mf_in[i * P:(i + 1) * P, :])
                mf_sb.append(mf_t)

            # ---- A^T in SBUF (bf16), via PE transposes ----
            at_sb = [work.tile([P, N], dt.bfloat16, tag=f"at{i}", name=f"at{i}") for i in range(NIT)]
            for jt in range(NJT):
                for half in range(2):
                    a_st = stage.tile([P, N // 2], dt.float32, tag="ast")
                    nc.sync.dma_start(
                        out=a_st,
                        in_=a_in[jt * P:(jt + 1) * P,
                                 half * (N // 2):(half + 1) * (N // 2)],
                    )
                    for q in range(NIT // 2):
                        it = half * (NIT // 2) + q
                        t_ps = psm.tile([P, P], dt.float32, tag="m")
                        nc.tensor.transpose(
                            t_ps, a_st[:, q * P:(q + 1) * P], ident
                        )
                        nc.any.tensor_copy(
                            out=at_sb[it][:, jt * P:(jt + 1) * P], in_=t_ps
                        )

            # ---- accumulators ----
            acc = [work.tile([P, NCOL], dt.float32, tag=f"acc{j}", name=f"acc{j}") for j in range(NJT)]
            for j in range(NJT):
                nc.any.memset(acc[j], 0.0)

            # ---- main loop over k chunks ----
            for kb in range(NKB):
                # Hrow broadcast: HK[p, (k,f)] = Hsh[kb*KB+k, f] for all p
                hflat = tmp.tile([1, NCOL], dt.float32, tag="hflat")
                nc.sync.dma_start(
                    out=hflat, in_=hsh_dram[0:1, kb * NCOL:(kb + 1) * NCOL]
                )
                hk_ps = psm.tile([P, NCOL], dt.float32, tag="m")
                nc.tensor.matmul(hk_ps, ones, hflat, start=True, stop=True)
                hk = hkp.tile([P, NCOL], dt.float32, tag="hk")
                nc.any.tensor_copy(out=hk, in_=hk_ps)

                # R tiles for this chunk
                r_kb = []
                for it in range(NIT):
                    r_t = rp.tile([P, NCOL], dt.bfloat16, tag=f"r{it}")
                    h_b = h_sb[it][:, :].unsqueeze(1).to_broadcast([P, KB, F])
                    mh_b = (
                        mh_sb[it][:, kb * KB:(kb + 1) * KB]
                        .unsqueeze(2)
                        .to_broadcast([P, KB, F])
                    )
                    r_view = r_t[:, :].rearrange("p (k f) -> p k f", k=KB)
                    nc.vector.tensor_mul(r_view, h_b, mh_b)
                    r_kb.append(r_t)

                for jt in range(NJT):
                    g_ps = psg.tile([P, NCOL], dt.float32, tag="g")
                    for it in range(NIT):
                        nc.tensor.matmul(
                            g_ps,
                            at_sb[it][:, jt * P:(jt + 1) * P],
                            r_kb[it],
                            start=(it == 0),
                            stop=(it == NIT - 1),
                        )
                    # epilogue: acc[jt] += g * mf[:,k] * hk
                    t1 = tmp.tile([P, NCOL], dt.float32, tag="t1")
                    mf_b = (
                        mf_sb[jt][:, kb * KB:(kb + 1) * KB]
                        .unsqueeze(2)
                        .to_broadcast([P, KB, F])
                    )
                    nc.vector.tensor_mul(
                        t1[:, :].rearrange("p (k f) -> p k f", k=KB),
                        g_ps[:, :].rearrange("p (k f) -> p k f", k=KB),
                        mf_b,
                    )
                    t2 = tmp.tile([P, NCOL], dt.float32, tag="t2")
                    nc.vector.tensor_mul(t2, t1, hk)
                    nc.vector.tensor_add(acc[jt], acc[jt], t2)

            # ---- finale: reduce k-chunk slots, store ----
            for jt in range(NJT):
                red = stage.tile([P, F], dt.float32, tag="red")
                nc.vector.tensor_reduce(
                    red,
                    acc[jt][:, :].rearrange("p (k f) -> p f k", k=KB),
                    axis=mybir.AxisListType.X,
                    op=mybir.AluOpType.add,
                )
                nc.sync.dma_start(out=out_p[jt * P:(jt + 1) * P, :], in_=red)

    nc.finalize()
    return nc


def _get_nc():
    if "nc" not in _CACHE:
        _CACHE["nc"] = _build()
    return _CACHE["nc"]


def _in_maps(node_features, adjacency_matrix, mask_father, mask_hadamard,
             weight, bias):
    x = np.ascontiguousarray(node_features, dtype=np.float32)
    a = np.ascontiguousarray(adjacency_matrix, dtype=np.float32)
    mf = np.ascontiguousarray(mask_father, dtype=np.float32)
    mh = np.ascontiguousarray(mask_hadamard, dtype=np.float32)
    w = np.ascontiguousarray(weight, dtype=np.float32)
    b = np.ascontiguousarray(bias, dtype=np.float32).reshape(1, F)
    maps = []
    for c in range(NCORES):
        s = slice(c * KSH, (c + 1) * KSH)
        maps.append({
            "x": x,
            "xs": np.ascontiguousarray(x[s, :]),
            "w": w,
            "b": b,
            "a": a,
            "mh": np.ascontiguousarray(mh[:, s]),
            "mf": np.ascontiguousarray(mf[:, s]),
        })
    return maps


def run_spmd(inputs, **kw):
    """Run the SPMD kernel; returns (summed_output, BassKernelResults)."""
    nc = _get_nc()
    maps = _in_maps(**inputs)
    res = run_bass_kernel_spmd(nc, maps, list(range(NCORES)), **kw)
    out = np.zeros((N, F), dtype=np.float32)
    for c in range(NCORES):
        out += res.results[c]["out_p"]
    return out, res


def kernel(node_features, adjacency_matrix, mask_father, mask_hadamard,
           weight, bias):
    out, _ = run_spmd(dict(
        node_features=node_features,
        adjacency_matrix=adjacency_matrix,
        mask_father=mask_father,
        mask_hadamard=mask_hadamard,
        weight=weight,
        bias=bias,
    ))
    return out
